# revision 30
# baseline (speedup 1.0000x reference)
"""Trainium2 Bass kernel for a dense transformer block (B=2, T=2048, C=1024, 16 heads).

Strategy (SPMD over 8 cores, one AllGather per group of 4):
  - cores 0-3 handle batch 0, cores 4-7 batch 1 (4 cores per sequence)
  - core with j = core%4 owns two 256-token query chunks: A at 256*j and
    B at 256*(7-j).  Each core computes LN1 + q/k/v ONLY for its own 512
    tokens; k and v (bf16, bias and LN-mean corrections applied) are
    AllGathered across the 4-core sequence group via an HBM bounce
    buffer.  Gathered token order is the ownership permutation
    [c0:A,B | c1:A,B | c2:A,B | c3:A,B]; causality is enforced with
    host-provided multiplicative masks in that permuted order, so the
    instruction stream stays identical on all cores.
  - attention: arm A (low chunk) covers the 8 chunk-A k-slots, arm B
    (high chunk) covers all 16 slots; masks kill invalid/diagonal parts.
  - q is computed while the collective runs; k/v readback is pipelined
    so attention starts as soon as the first head-pair's k has landed.

kernel(**inputs) takes the full unsharded inputs and returns the full
[2, 2048, 1024] output.
"""
import numpy as np
import ml_dtypes

import concourse.bass as bass
import concourse.tile as tile
from concourse import bacc, mybir
from concourse.bass_utils import run_bass_kernel_spmd

BF16 = ml_dtypes.bfloat16
F32 = mybir.dt.float32
F32R = mybir.dt.float32r
DBF = mybir.dt.bfloat16

C = 1024          # embed dim
T = 2048          # seq len
B = 2
H = 16            # heads
D = 64            # head dim
HID = 4096
NC = 8            # cores
CH = C // 128     # 8 channel chunks
QCH = 256         # query chunk width
KT_A = 8          # k-tiles for arm A
KT_B = 16         # k-tiles for arm B
LN_EPS = 1e-5
ATT_SCALE = 1.0 / 8.0   # 1/sqrt(64)

# arm A processes the 8 chunk-A k-slots of the gathered (permuted) kv:
# slot 4*r + ti holds tokens [256*r + 128*ti, +128) of owner core r.
SLOTA = [4 * r + ti for r in range(4) for ti in range(2)]

_BUILD_CACHE = {}


def r32(ap):
    """View an f32 access pattern as float32r for full-rate matmuls."""
    return ap.bitcast(F32R)


def f32(ap):
    """View a float32r access pattern as plain f32 for vector-engine ops."""
    return ap.bitcast(F32)


def build_kernel(reps=1, skip=(), ag_split=True, G=2):
    key = (reps, tuple(skip), ag_split, G)
    if key in _BUILD_CACHE:
        return _BUILD_CACHE[key]
    nc = bacc.Bacc("TRN2", target_bir_lowering=False, debug=False, num_devices=NC)

    # ---- I/O ----
    x_own_t = nc.dram_tensor("x_own", [CH, 128, 512], F32R, kind="ExternalInput")
    wq_t = nc.dram_tensor("wq", [8, 128, CH, 128], DBF, kind="ExternalInput")
    wk_t = nc.dram_tensor("wk", [8, 128, CH, 128], DBF, kind="ExternalInput")
    wv_t = nc.dram_tensor("wv", [CH, 128, C], DBF, kind="ExternalInput")
    wp_t = nc.dram_tensor("wp", [8, 128, CH, 128], F32R, kind="ExternalInput")
    w1_t = nc.dram_tensor("w1", [32, 128, CH, 128], F32R, kind="ExternalInput")
    w2_t = nc.dram_tensor("w2", [8, 128, 32, 128], DBF, kind="ExternalInput")
    bq_t = nc.dram_tensor("bq", [128, 8], F32, kind="ExternalInput")
    bk_t = nc.dram_tensor("bk", [128, 8], F32, kind="ExternalInput")
    bv_t = nc.dram_tensor("bv", [1, C], DBF, kind="ExternalInput")
    bp_t = nc.dram_tensor("bp", [128, 8], F32, kind="ExternalInput")
    b1_t = nc.dram_tensor("b1", [128, 32], F32, kind="ExternalInput")
    b2_t = nc.dram_tensor("b2", [128, 8], F32, kind="ExternalInput")
    cq_t = nc.dram_tensor("cq", [1, C], DBF, kind="ExternalInput")
    ck_t = nc.dram_tensor("ck", [1, C], DBF, kind="ExternalInput")
    cv_t = nc.dram_tensor("cv", [1, C], DBF, kind="ExternalInput")
    c1_t = nc.dram_tensor("c1", [1, HID], DBF, kind="ExternalInput")
    mask_t = nc.dram_tensor("mask", [128, 8, 4, QCH], DBF, kind="ExternalInput")
    out_t = nc.dram_tensor("out", [CH, 128, 512], F32, kind="ExternalOutput")

    with tile.TileContext(nc) as tc, nc.allow_low_precision(
            reason="float32r operands feeding full-rate matmuls"):
        def body(it):
            with (
                tc.tile_pool(name=f"const{it}", bufs=1) as const,
                tc.tile_pool(name=f"bigp{it}", bufs=1) as bigp,
                tc.tile_pool(name=f"wpool{it}", bufs=2) as wpool,
                tc.tile_pool(name=f"stat{it}", bufs=1) as stat,
                tc.tile_pool(name=f"tmp{it}", bufs=2) as tmp,
                tc.tile_pool(name=f"dram{it}", bufs=1, space="DRAM") as dram,
            ):
                def do(name):
                    return name not in skip

                ones_f = const.tile([128, 128], F32)
                nc.vector.memset(ones_f, 1.0)
                ones_col = const.tile([128, 1], F32R)
                nc.vector.tensor_copy(ones_col, ones_f[:, 0:1])
                ones_row = const.tile([1, 128], F32R)
                nc.vector.tensor_copy(ones_row, ones_f[0:1, :])
                ones_row_bf = const.tile([1, 128], DBF)
                nc.vector.tensor_copy(ones_row_bf, ones_f[0:1, :])
                eps_c = const.tile([1, 1], F32)
                nc.vector.memset(eps_c, LN_EPS)

                def cload(name, t, shape, dtype=F32):
                    s = const.tile(shape, dtype, name=name)
                    nc.sync.dma_start(out=s, in_=t[tuple(slice(None) for _ in shape)])
                    return s

                cq_sb = const.tile([1, C], DBF, tag="csum_a", name="cqs")
                nc.sync.dma_start(out=cq_sb, in_=cq_t[:, :])
                ckv_sb = const.tile([1, 2 * C], DBF, name="ckvs")
                nc.sync.dma_start(out=ckv_sb[:, 0:C], in_=ck_t[:, :])
                nc.sync.dma_start(out=ckv_sb[:, C:2 * C], in_=cv_t[:, :])
                ck_sb = ckv_sb[:, 0:C]
                cv_pack = ckv_sb[:, C:2 * C]
                bq_sb = cload("bqs", bq_t, [128, 8])
                bk_sb = cload("bks", bk_t, [128, 8])
                bp_sb = cload("bps", bp_t, [128, 8])
                b1_sb = cload("b1s", b1_t, [128, 32])
                b2_sb = cload("b2s", b2_t, [128, 8])
                bvrow_sb = cload("bvrs", bv_t, [1, C], DBF)
                mask_sb = cload("masks", mask_t, [128, 8, 4, QCH], DBF)

                x_own_sb = bigp.tile([128, CH, 512], F32R, tag="xown")
                for ci in range(CH):
                    nc.sync.dma_start(out=x_own_sb[:, ci, :], in_=x_own_t[ci, :, :])

                def ln_group(x_ap, w, out_ap_fn, ps_ln, gi, ma_ap):
                    """Pseudo-LayerNorm of one <=512-token group: writes
                    out = x * rstd (per token) and ma_ap[0, :w] = mu * rstd.
                    The mean subtraction is applied later as a rank-1
                    correction matmul; LN gain/bias are folded into the
                    following weights on the host."""
                    ps_s = ps_ln.tile([1, 512], F32, tag="s", bufs=2, name=f"pss_{it}_{gi}")
                    ps_q = ps_ln.tile([1, 512], F32, tag="q", bufs=2, name=f"psq_{it}_{gi}")
                    for ci in range(CH):
                        nc.tensor.matmul(
                            ps_s[:, :w], ones_col, x_ap[:, ci, :],
                            start=(ci == 0), stop=(ci == CH - 1))
                    for ci in range(CH):
                        sq = tmp.tile([128, 512], F32R, tag="sq", bufs=1,
                                      name=f"sq_{it}_{gi}_{ci}")
                        nc.scalar.activation(sq[:, :w], f32(x_ap[:, ci, :]),
                                             mybir.ActivationFunctionType.Square)
                        nc.tensor.matmul(
                            ps_q[:, :w], ones_col, sq[:, :w],
                            start=(ci == 0), stop=(ci == CH - 1))
                    mu = stat.tile([1, 512], F32, tag="mu", name=f"mu_{it}_{gi}")
                    nc.vector.tensor_scalar_mul(mu[:, :w], ps_s[:, :w], 1.0 / C)
                    ex2 = stat.tile([1, 512], F32, tag="ex2", name=f"ex2_{it}_{gi}")
                    nc.vector.tensor_scalar_mul(ex2[:, :w], ps_q[:, :w], 1.0 / C)
                    var = stat.tile([1, 512], F32, tag="var", name=f"var_{it}_{gi}")
                    nc.scalar.activation(var[:, :w], mu[:, :w],
                                         mybir.ActivationFunctionType.Square)
                    nc.vector.tensor_sub(var[:, :w], ex2[:, :w], var[:, :w])
                    nc.scalar.activation(var[:, :w], var[:, :w],
                                         mybir.ActivationFunctionType.Sqrt,
                                         bias=eps_c[:, :])
                    rstd = stat.tile([1, 512], F32R, tag="ex2", name=f"rstd_{it}_{gi}")
                    nc.vector.reciprocal(rstd[:, :w], var[:, :w])
                    nc.vector.tensor_mul(ma_ap[:, :w], mu[:, :w], f32(rstd[:, :w]))
                    psb_rs = ps_ln.tile([128, 512], F32, tag="brs", bufs=2,
                                        name=f"brs_{it}_{gi}")
                    nc.tensor.matmul(psb_rs[:, :w], ones_row, rstd[:, :w],
                                     start=True, stop=True)
                    for ci in range(CH):
                        nc.vector.tensor_mul(out_ap_fn(ci), f32(x_ap[:, ci, :]),
                                             psb_rs[:, :w])

                h1o = bigp.tile([128, CH, 512], DBF, tag="h1o")
                q_sb = bigp.tile([128, 8, 512], DBF, tag="qsb")
                k_sb = bigp.tile([128, 8, T], DBF, tag="ksb")
                v_sb = bigp.tile([128, 16, H, D + 1], DBF, tag="vsb")
                nc.vector.memset(v_sb[:, :, :, D:D + 1], 1.0)
                if 'kv' in skip:
                    nc.vector.memset(k_sb, 0.01)
                    nc.vector.memset(v_sb[:, :, :, 0:D], 0.01)

                # HBM bounce buffers for the k/v AllGather (bf16).
                # k rows [0,1024): k feature-major [1024 feat, 512 tok]
                # v rows: v token-major [512 tok, 1024 feat], two rows/token.
                # ag_split=False packs both into one buffer (one collective
                # measured cheaper than two back-to-back on this fabric).
                if ag_split:
                    k_bin = dram.tile([1024, 512], DBF)
                    k_bout = dram.tile([4096, 512], DBF)
                    v_bin = dram.tile([1024, 512], DBF)
                    v_bout = dram.tile([4096, 512], DBF)
                    v_rows0 = 0          # v offset within its bounce
                    k_stride, v_stride = 1024, 1024
                else:
                    kv_in = dram.tile([2048, 512], DBF)
                    kv_gather = dram.tile([4 * 2048, 512], DBF)
                    k_bin = kv_in
                    k_bout = kv_gather
                    v_bin = kv_in
                    v_bout = kv_gather
                    v_rows0 = 1024
                    k_stride, v_stride = 2048, 2048

                with (
                    tc.tile_pool(name=f"psln{it}", bufs=1, space="PSUM") as ps_ln,
                    tc.tile_pool(name=f"psacc{it}", bufs=2, space="PSUM") as ps_acc,
                ):
                    # ---- LN1 over own tokens -> h1o ----
                    ma_own = const.tile([1, 512], DBF, name="ma_own")
                    if do('lnown'):
                        ln_group(x_own_sb, 512,
                                 lambda ci: h1o[:, ci, :], ps_ln, "own", ma_own)
                    else:
                        nc.vector.memset(h1o.rearrange('p a b -> p (a b)'), 0.01)
                        nc.vector.memset(ma_own, 0.01)

                    # ---- k for own tokens -> bounce ----
                    k_own = bigp.tile([128, 8, 512], DBF, tag="kown")
                    for mt in ([] if 'kv' in skip else range(8)):
                        wk_sb = wpool.tile([128, CH, 128], DBF, tag="w",
                                           name=f"wk_{it}_{mt}")
                        nc.sync.dma_start(out=wk_sb, in_=wk_t[mt, :, :, :])
                        ps = ps_acc.tile([128, 512], F32, tag="acc",
                                         name=f"psk_{it}_{mt}")
                        for ci in range(CH):
                            nc.tensor.matmul(ps, wk_sb[:, ci, :], h1o[:, ci, :],
                                             start=(ci == 0), stop=False)
                        nc.tensor.matmul(ps, ck_sb[:, 128 * mt:128 * mt + 128],
                                         ma_own, start=False, stop=True)
                        nc.scalar.activation(k_own[:, mt, :], ps,
                                             mybir.ActivationFunctionType.Identity,
                                             bias=bk_sb[:, mt:mt + 1])
                        nc.sync.dma_start(out=k_bin[128 * mt:128 * mt + 128, :],
                                          in_=k_own[:, mt, :])
                    # ---- AllGather k as soon as it is ready (split mode) ----
                    if do('kv') and do('ag') and ag_split:
                        nc.gpsimd.collective_compute(
                            "AllGather",
                            mybir.AluOpType.bypass,
                            replica_groups=[[0, 1, 2, 3], [4, 5, 6, 7]],
                            ins=[k_bin.opt()],
                            outs=[k_bout.opt()],
                        )

                    # ---- v for own tokens (token-major) -> bounce ----
                    wv_sb = bigp.tile([128, CH, C], DBF, tag="wv")
                    if do('kv'):
                        for ci in range(CH):
                            nc.sync.dma_start(out=wv_sb[:, ci, :], in_=wv_t[ci, :, :])
                    v_own = bigp.tile([128, 4, 2, 512], DBF, tag="vown")
                    for tt in ([] if 'kv' in skip else range(4)):
                        for vh in range(2):
                            ps = ps_acc.tile([128, 512], F32, tag="acc",
                                             name=f"psv_{it}_{tt}_{vh}")
                            for ci in range(CH):
                                nc.tensor.matmul(
                                    ps, h1o[:, ci, 128 * tt:128 * tt + 128],
                                    wv_sb[:, ci, 512 * vh:512 * vh + 512],
                                    start=(ci == 0), stop=False)
                            nc.tensor.matmul(
                                ps, ma_own[:, 128 * tt:128 * tt + 128],
                                cv_pack[:, 512 * vh:512 * vh + 512],
                                start=False, stop=False)
                            nc.tensor.matmul(
                                ps, ones_row_bf,
                                bvrow_sb[:, 512 * vh:512 * vh + 512],
                                start=False, stop=True)
                            nc.scalar.activation(
                                v_own[:, tt, vh, :], ps,
                                mybir.ActivationFunctionType.Copy)
                            dst = v_bin[v_rows0 + 256 * tt:v_rows0 + 256 * (tt + 1), :] \
                                .rearrange("(p two) c -> p two c", two=2)[:, vh, :]
                            nc.sync.dma_start(out=dst, in_=v_own[:, tt, vh, :])

                    # ---- AllGather v (split) or fused k+v ----
                    if do('kv') and do('ag'):
                        nc.gpsimd.collective_compute(
                            "AllGather",
                            mybir.AluOpType.bypass,
                            replica_groups=[[0, 1, 2, 3], [4, 5, 6, 7]],
                            ins=[v_bin.opt()],
                            outs=[v_bout.opt()],
                        )

                    # ---- q for own tokens (overlaps the collective) ----
                    for mt in (range(8) if do('q') else []):
                        wq_sb = wpool.tile([128, CH, 128], DBF, tag="w",
                                           name=f"wq_{it}_{mt}")
                        nc.sync.dma_start(out=wq_sb, in_=wq_t[mt, :, :, :])
                        ps = ps_acc.tile([128, 512], F32, tag="acc",
                                         name=f"psq2_{it}_{mt}")
                        for ci in range(CH):
                            nc.tensor.matmul(ps, wq_sb[:, ci, :], h1o[:, ci, :],
                                             start=(ci == 0), stop=False)
                        nc.tensor.matmul(ps, cq_sb[:, 128 * mt:128 * mt + 128],
                                         ma_own, start=False, stop=True)
                        nc.scalar.activation(q_sb[:, mt, :], ps,
                                             mybir.ActivationFunctionType.Identity,
                                             bias=bq_sb[:, mt:mt + 1])
                    if 'q' in skip:
                        nc.vector.memset(q_sb.rearrange('p a b -> p (a b)'), 0.01)

                    # ---- readback of gathered k/v ----
                    if do('kv') and do('rb'):
        # k for the first head-pair, then all v (AV needs it a
                        # few microseconds after the first scores), then the
                        # remaining k head-pairs.
                        def k_rb(mt):
                            for r in range(4):
                                nc.sync.dma_start(
                                    out=k_sb[:, mt, 512 * r:512 * (r + 1)],
                                    in_=k_bout[k_stride * r + 128 * mt:
                                               k_stride * r + 128 * (mt + 1), :])
                        k_rb(0)
                        for r in range(4):
                            for si in range(4):
                                s = 4 * r + si
                                src = v_bout[
                                    v_stride * r + v_rows0 + 256 * si:
                                    v_stride * r + v_rows0 + 256 * (si + 1), :] \
                                    .rearrange("(p two) c -> p (two c)", two=2) \
                                    .rearrange("p (h d) -> p h d", h=H)
                                nc.sync.dma_start(out=v_sb[:, s, :, 0:D], in_=src)
                        for mt in range(1, 8):
                            k_rb(mt)

                # ---------- attention ----------
                y_sb = bigp.tile([128, 8, 512], F32R, tag="wv")
                with (
                    tc.tile_pool(name=f"psscr{it}", bufs=(2 if G == 2 else 1),
                                 space="PSUM") as ps_scr,
                    tc.tile_pool(name=f"psy{it}", bufs=1, space="PSUM") as ps_y,
                    tc.tile_pool(name=f"psbd{it}", bufs=2, space="PSUM") as ps_bd,
                    tc.tile_pool(name=f"esp{it}", bufs=(6 if G == 2 else 1)) as espool,
                ):
                    att_rts = [] if 'att' in skip else list(range(8))
                    if 'att' in skip:
                        nc.vector.tensor_copy(y_sb.rearrange('p a b -> p (a b)'), ones_f[:, 0:1].broadcast_to((128, CH * 512)))
                    for rt in att_rts:
                        for arm, (nkt, q0) in enumerate([(KT_A, 0), (KT_B, QCH)]):
                            hh = [2 * rt, 2 * rt + 1]
                            psYs = [ps_y.tile([D + 1, QCH], F32, tag=f"y{u}",
                                              name=f"psY_{it}_{rt}_{arm}_{u}")
                                    for u in range(2)]
                            # 2-tile groups, both head-pair members (u) in ONE
                            # shared PSUM tile -> one exp call and at most one
                            # mask call per group (halves ACT/DVE call count).
                            # PSUM tiles rotate (bufs=2) so group g+1's scores
                            # run while group g's exp/AV drain.
                            for g in range(nkt // 2):
                                pall = ps_scr.tile([128, 4, QCH], F32, tag="scr",
                                                   name=f"psS_{it}_{rt}_{arm}_{g}")
                                eall = espool.tile([128, 4, QCH], DBF, tag="es",
                                                   name=f"es_{it}_{rt}_{arm}_{g}")
                                for i in range(2):
                                    kt = 2 * g + i
                                    slot = SLOTA[kt] if arm == 0 else kt
                                    for u in range(2):
                                        po = 64 * u
                                        nc.tensor.matmul(
                                            pall[:, 2 * u + i, :],
                                            k_sb[po:po + 64, rt,
                                                 128 * slot:128 * slot + 128],
                                            q_sb[po:po + 64, rt, q0:q0 + QCH],
                                            start=True, stop=True)
                                nc.scalar.activation(
                                    eall, pall,
                                    mybir.ActivationFunctionType.Exp,
                                    scale=ATT_SCALE)
                                # arm A: mask every group (mask slot g);
                                # arm B: only odd groups (chunk-B tiles) can
                                # be invalid/diagonal (mask slot 4 + g//2)
                                if arm == 0:
                                    nc.vector.tensor_mul(
                                        eall, eall, mask_sb[:, g, :, :])
                                elif g % 2 == 1:
                                    nc.vector.tensor_mul(
                                        eall, eall, mask_sb[:, 4 + g // 2, :, :])
                                # AV for this group, interleaved so PE keeps
                                # busy while ACT runs the next group's exp
                                for u in range(2):
                                    for i in range(2):
                                        kt = 2 * g + i
                                        slot = SLOTA[kt] if arm == 0 else kt
                                        nc.tensor.matmul(
                                            psYs[u], v_sb[:, slot, hh[u], :],
                                            eall[:, 2 * u + i, :],
                                            start=(kt == 0),
                                            stop=(kt == nkt - 1),
                                            skip_group_check=True)
                            for u in range(2):
                                po = 64 * u
                                psY = psYs[u]
                                rd = stat.tile([1, QCH], F32R, tag="mu",
                                               name=f"rd_{it}_{rt}_{arm}_{u}")
                                nc.vector.reciprocal(rd, psY[D:D + 1, :])
                                psD = ps_bd.tile([64, QCH], F32, tag="bd",
                                                 name=f"psD_{it}_{rt}_{arm}_{u}")
                                nc.tensor.matmul(psD, ones_row[:, 0:64], rd,
                                                 start=True, stop=True)
                                rdb = tmp.tile([64, QCH], F32, tag="rdb", bufs=1,
                                               name=f"rdb_{it}_{rt}_{arm}_{u}")
                                nc.scalar.activation(rdb, psD,
                                                     mybir.ActivationFunctionType.Copy)
                                nc.vector.tensor_mul(
                                    y_sb[po:po + 64, rt, q0:q0 + QCH],
                                    psY[0:D, :], rdb)

                # ---------- proj + residual -> x2, LN2, MLP ----------
                x2_sb = bigp.tile([128, CH, 512], F32R, tag="x2")
                h2_sb = bigp.tile([128, CH, 512], F32R, tag="wv")
                hm_sb = bigp.tile([128, 32, 512], DBF, tag="ksb")
                out_sb = bigp.tile([128, CH, 512], F32, tag="vsb")
                with (
                    tc.tile_pool(name=f"psln2{it}", bufs=1, space="PSUM") as ps_ln2,
                    tc.tile_pool(name=f"psacc2{it}", bufs=2, space="PSUM") as ps_acc2,
                ):
                    for mt in (range(8) if do('proj') else []):
                        wp_sb = wpool.tile([128, CH, 128], F32R, tag="w",
                                           name=f"wp_{it}_{mt}")
                        nc.sync.dma_start(out=wp_sb, in_=wp_t[mt, :, :, :])
                        ps = ps_acc2.tile([128, 512], F32, tag="acc",
                                          name=f"psp_{it}_{mt}")
                        for ci in range(CH):
                            nc.tensor.matmul(ps, wp_sb[:, ci, :],
                                             y_sb[:, ci, :],
                                             start=(ci == 0), stop=(ci == CH - 1))
                        nc.vector.scalar_tensor_tensor(
                            out=x2_sb[:, mt, :], in0=ps, scalar=bp_sb[:, mt:mt + 1],
                            in1=f32(x_own_sb[:, mt, :]),
                            op0=mybir.AluOpType.add, op1=mybir.AluOpType.add)
                    if 'proj' in skip:
                        for mt in range(8):
                            nc.vector.tensor_copy(x2_sb[:, mt, :], x_own_sb[:, mt, :])

                    c1_sb = const.tile([1, HID], DBF, tag="csum_a", name="c1s")
                    nc.sync.dma_start(out=c1_sb, in_=c1_t[:, :])
                    ma_x2 = const.tile([1, 512], DBF, name="ma_x2")
                    if do('ln2'):
                        ln_group(x2_sb, 512,
                                 lambda ci: h2_sb[:, ci, :], ps_ln2, "ln2", ma_x2)
                    else:
                        nc.vector.memset(
                            f32(h2_sb.rearrange('p a b -> p (a b)')), 0.01)
                        nc.vector.memset(ma_x2, 0.01)

                    mlp_hts = [] if 'mlp' in skip else list(range(32))
                    if 'mlp' in skip:
                        nc.vector.memset(hm_sb, 0.01)
                    for ht in mlp_hts:
                        w1_sb = wpool.tile([128, CH, 128], F32R, tag="w",
                                           name=f"w1_{it}_{ht}")
                        nc.sync.dma_start(out=w1_sb, in_=w1_t[ht, :, :, :])
                        ps = ps_acc2.tile([128, 512], F32, tag="acc",
                                          name=f"psm1_{it}_{ht}")
                        for ci in range(CH):
                            nc.tensor.matmul(ps, w1_sb[:, ci, :],
                                             h2_sb[:, ci, :],
                                             start=(ci == 0), stop=False)
                        nc.tensor.matmul(ps, c1_sb[:, 128 * ht:128 * ht + 128],
                                         ma_x2, start=False, stop=True)
                        nc.scalar.activation(hm_sb[:, ht, :], ps,
                                             mybir.ActivationFunctionType.Gelu,
                                             bias=b1_sb[:, ht:ht + 1])

                    for mt in range(8):
                        ps = ps_acc2.tile([128, 512], F32, tag="acc",
                                          name=f"psm2_{it}_{mt}")
                        for qq in range(4):
                            w2_sb = wpool.tile([128, 8, 128], DBF, tag="w",
                                               name=f"w2_{it}_{mt}_{qq}")
                            nc.sync.dma_start(out=w2_sb,
                                              in_=w2_t[mt, :, 8 * qq:8 * qq + 8, :])
                            for hc in range(8):
                                g = 8 * qq + hc
                                nc.tensor.matmul(ps, w2_sb[:, hc, :], hm_sb[:, g, :],
                                                 start=(g == 0), stop=(g == 31))
                        nc.vector.scalar_tensor_tensor(
                            out=out_sb[:, mt, :], in0=ps, scalar=b2_sb[:, mt:mt + 1],
                            in1=f32(x2_sb[:, mt, :]),
                            op0=mybir.AluOpType.add, op1=mybir.AluOpType.add)
                    for mt in range(8):
                        nc.sync.dma_start(out=out_t[mt, :, :], in_=out_sb[:, mt, :])

        # Collectives deadlock inside a rolled For_i loop, so repetitions
        # (used only for timing) are unrolled in Python.
        for it in range(reps):
            body(it)

    nc.compile()
    _BUILD_CACHE[key] = nc
    return nc


def _tile_w(w, n_chunks, n_mt):
    """[K, M] -> [n_mt, 128, n_chunks, 128] so each lhsT tile is contiguous."""
    return np.ascontiguousarray(
        w.reshape(n_chunks, 128, n_mt, 128).transpose(2, 1, 0, 3))


def _col8(v):
    """[N*128] -> [128, N] per-partition scalar table."""
    n = v.shape[0] // 128
    return np.ascontiguousarray(v.reshape(n, 128).T)


def slot_pos(s):
    """Start position of gathered k/v slot s (permuted ownership order)."""
    r, rem = divmod(s, 4)
    if rem < 2:
        return 256 * r + 128 * rem            # chunk A of core r
    return 256 * (7 - r) + 128 * (rem - 2)    # chunk B of core r


def make_in_maps(x, w_qkv, b_qkv, w_proj, b_proj, ln1_g, ln1_b, ln2_g, ln2_b,
                 w1, b1, w2, b2):
    f = lambda a: np.asarray(a, np.float32)
    x = f(x)
    w_qkv, b_qkv, w_proj, b_proj = f(w_qkv), f(b_qkv), f(w_proj), f(b_proj)
    w1, b1, w2, b2 = f(w1), f(b1), f(w2), f(b2)
    wq, wk, wv = w_qkv[:, 0:C], w_qkv[:, C:2 * C], w_qkv[:, 2 * C:3 * C]
    # fold LN1 gain into qkv weights and LN1 bias into qkv biases; the
    # per-token mean subtraction becomes a rank-1 correction with the
    # negated column sums (cq/ck/cv).  Same for LN2 into w1/b1.
    wq_e = wq * ln1_g[:, None]
    wk_e = wk * ln1_g[:, None]
    wv_e = wv * ln1_g[:, None]
    bq_e = b_qkv[0:C] + wq.T @ ln1_b
    bk_e = b_qkv[C:2 * C] + wk.T @ ln1_b
    bv_e = b_qkv[2 * C:3 * C] + wv.T @ ln1_b
    w1_e = w1 * ln2_g[:, None]
    b1_e = b1 + w1.T @ ln2_b
    common = {
        "wq": _tile_w(wq_e, CH, 8).astype(BF16),
        "wk": _tile_w(wk_e, CH, 8).astype(BF16),
        "wv": np.ascontiguousarray(wv_e.reshape(CH, 128, C)).astype(BF16),
        "wp": _tile_w(w_proj, CH, 8),
        "w1": _tile_w(w1_e, CH, 32),
        "w2": _tile_w(w2, 32, 8).astype(BF16),
        "bq": _col8(bq_e), "bk": _col8(bk_e),
        "bv": np.ascontiguousarray(bv_e.reshape(1, C)).astype(BF16),
        "bp": _col8(b_proj), "b1": _col8(b1_e), "b2": _col8(b2),
        "cq": (-wq_e.sum(axis=0).reshape(1, C)).astype(BF16),
        "ck": (-wk_e.sum(axis=0).reshape(1, C)).astype(BF16),
        "cv": (-wv_e.sum(axis=0).reshape(1, C)).astype(BF16),
        "c1": (-w1_e.sum(axis=0).reshape(1, HID)).astype(BF16),
    }
    in_maps = []
    kk = np.arange(128)
    qq = np.arange(QCH)
    for c in range(NC):
        seq = c // 4
        j = c % 4
        xf = np.ascontiguousarray(x[seq].T)          # [C, T] feature-major
        qa0, qb0 = QCH * j, QCH * (7 - j)
        x_own = np.concatenate([xf[:, qa0:qa0 + QCH], xf[:, qb0:qb0 + QCH]], axis=1)
        # masks over the gathered (permuted) slot order:
        # rows 0..7: arm A vs the 8 chunk-A slots (SLOTA order)
        # rows 8..15: arm B vs the 8 chunk-B slots (4r+2+ti order)
        masks = np.zeros((16, 128, QCH), np.float32)
        for kt in range(8):
            pos0 = slot_pos(SLOTA[kt])
            masks[kt] = (qa0 + qq[None, :]) >= (pos0 + kk[:, None])
        for mr in range(8):
            r, ti = divmod(mr, 2)
            pos0 = slot_pos(4 * r + 2 + ti)
            masks[8 + mr] = (qb0 + qq[None, :]) >= (pos0 + kk[:, None])
        # expand for the merged-u exp: slot g holds the group's two mask
        # rows duplicated for both head-pair members ((u,i) = 2u+i order).
        mx = np.zeros((8, 4, 128, QCH), np.float32)
        for g in range(4):                       # arm A groups
            mx[g, 0] = mx[g, 2] = masks[2 * g]
            mx[g, 1] = mx[g, 3] = masks[2 * g + 1]
        for gb in range(4):                      # arm B odd groups
            mx[4 + gb, 0] = mx[4 + gb, 2] = masks[8 + 2 * gb]
            mx[4 + gb, 1] = mx[4 + gb, 3] = masks[8 + 2 * gb + 1]
        m = {
            "x_own": np.ascontiguousarray(x_own.reshape(CH, 128, 512)),
            "mask": np.ascontiguousarray(mx.transpose(2, 0, 1, 3)).astype(BF16),
        }
        m.update(common)
        in_maps.append(m)
    return in_maps


def assemble_output(results):
    out = np.zeros((B, T, C), np.float32)
    for c in range(NC):
        seq = c // 4
        j = c % 4
        yf = results[c]["out"].reshape(C, 512)       # feature-major [C, 512]
        qa0, qb0 = QCH * j, QCH * (7 - j)
        out[seq, qa0:qa0 + QCH, :] = yf[:, 0:QCH].T
        out[seq, qb0:qb0 + QCH, :] = yf[:, QCH:2 * QCH].T
    return out


def kernel(**inputs):
    nc = build_kernel(reps=1)
    in_maps = make_in_maps(**inputs)
    res = run_bass_kernel_spmd(nc, in_maps, list(range(NC)))
    return assemble_output(res.results)


# revision 31
# speedup vs baseline: 1.0639x; 1.0639x over previous
"""Trainium2 Bass kernel for a dense transformer block (B=2, T=2048, C=1024, 16 heads).

Strategy (SPMD over 8 cores, one AllGather per group of 4):
  - cores 0-3 handle batch 0, cores 4-7 batch 1 (4 cores per sequence)
  - core with j = core%4 owns two 256-token query chunks: A at 256*j and
    B at 256*(7-j).  Each core computes LN1 + q/k/v ONLY for its own 512
    tokens; k and v (bf16, bias and LN-mean corrections applied) are
    AllGathered across the 4-core sequence group via an HBM bounce
    buffer.  Gathered token order is the ownership permutation
    [c0:A,B | c1:A,B | c2:A,B | c3:A,B]; causality is enforced with
    host-provided multiplicative masks in that permuted order, so the
    instruction stream stays identical on all cores.
  - attention: arm A (low chunk) covers the 8 chunk-A k-slots, arm B
    (high chunk) covers all 16 slots; masks kill invalid/diagonal parts.
  - q is computed while the collective runs; k/v readback is pipelined
    so attention starts as soon as the first head-pair's k has landed.

kernel(**inputs) takes the full unsharded inputs and returns the full
[2, 2048, 1024] output.
"""
import numpy as np
import ml_dtypes

import concourse.bass as bass
import concourse.tile as tile
from concourse import bacc, mybir
from concourse.bass_utils import run_bass_kernel_spmd

BF16 = ml_dtypes.bfloat16
F32 = mybir.dt.float32
F32R = mybir.dt.float32r
DBF = mybir.dt.bfloat16

C = 1024          # embed dim
T = 2048          # seq len
B = 2
H = 16            # heads
D = 64            # head dim
HID = 4096
NC = 8            # cores
CH = C // 128     # 8 channel chunks
QCH = 256         # query chunk width
KT_A = 8          # k-tiles for arm A
KT_B = 16         # k-tiles for arm B
LN_EPS = 1e-5
ATT_SCALE = 1.0 / 8.0   # 1/sqrt(64)

# arm A processes the 8 chunk-A k-slots of the gathered (permuted) kv:
# slot 4*r + ti holds tokens [256*r + 128*ti, +128) of owner core r.
SLOTA = [4 * r + ti for r in range(4) for ti in range(2)]

_BUILD_CACHE = {}


def r32(ap):
    """View an f32 access pattern as float32r for full-rate matmuls."""
    return ap.bitcast(F32R)


def f32(ap):
    """View a float32r access pattern as plain f32 for vector-engine ops."""
    return ap.bitcast(F32)


def build_kernel(reps=1, skip=(), ag_split=True, G=2):
    key = (reps, tuple(skip), ag_split, G)
    if key in _BUILD_CACHE:
        return _BUILD_CACHE[key]
    nc = bacc.Bacc("TRN2", target_bir_lowering=False, debug=False, num_devices=NC)

    # ---- I/O ----
    x_own_t = nc.dram_tensor("x_own", [CH, 128, 512], F32R, kind="ExternalInput")
    wq_t = nc.dram_tensor("wq", [8, 128, CH, 128], DBF, kind="ExternalInput")
    wk_t = nc.dram_tensor("wk", [8, 128, CH, 128], DBF, kind="ExternalInput")
    wv_t = nc.dram_tensor("wv", [CH, 128, C], DBF, kind="ExternalInput")
    wp_t = nc.dram_tensor("wp", [8, 128, CH, 128], F32R, kind="ExternalInput")
    w1_t = nc.dram_tensor("w1", [32, 128, CH, 128], F32R, kind="ExternalInput")
    w2_t = nc.dram_tensor("w2", [8, 128, 32, 128], DBF, kind="ExternalInput")
    bq_t = nc.dram_tensor("bq", [128, 8], F32, kind="ExternalInput")
    bk_t = nc.dram_tensor("bk", [128, 8], F32, kind="ExternalInput")
    bv_t = nc.dram_tensor("bv", [1, C], DBF, kind="ExternalInput")
    bp_t = nc.dram_tensor("bp", [128, 8], F32, kind="ExternalInput")
    b1_t = nc.dram_tensor("b1", [128, 32], F32, kind="ExternalInput")
    b2_t = nc.dram_tensor("b2", [128, 8], F32, kind="ExternalInput")
    cq_t = nc.dram_tensor("cq", [1, C], DBF, kind="ExternalInput")
    ck_t = nc.dram_tensor("ck", [1, C], DBF, kind="ExternalInput")
    cv_t = nc.dram_tensor("cv", [1, C], DBF, kind="ExternalInput")
    c1_t = nc.dram_tensor("c1", [1, HID], DBF, kind="ExternalInput")
    mask_t = nc.dram_tensor("mask", [128, 8, 4, QCH], DBF, kind="ExternalInput")
    out_t = nc.dram_tensor("out", [CH, 128, 512], F32, kind="ExternalOutput")

    with tile.TileContext(nc) as tc, nc.allow_low_precision(
            reason="float32r operands feeding full-rate matmuls"):
        def body(it):
            with (
                tc.tile_pool(name=f"const{it}", bufs=1) as const,
                tc.tile_pool(name=f"bigp{it}", bufs=1) as bigp,
                tc.tile_pool(name=f"wpool{it}", bufs=2) as wpool,
                tc.tile_pool(name=f"stat{it}", bufs=1) as stat,
                tc.tile_pool(name=f"tmp{it}", bufs=2) as tmp,
                tc.tile_pool(name=f"dram{it}", bufs=1, space="DRAM") as dram,
            ):
                def do(name):
                    return name not in skip

                ones_f = const.tile([128, 128], F32)
                nc.vector.memset(ones_f, 1.0)
                ones_col = const.tile([128, 1], F32R)
                nc.vector.tensor_copy(ones_col, ones_f[:, 0:1])
                ones_row = const.tile([1, 128], F32R)
                nc.vector.tensor_copy(ones_row, ones_f[0:1, :])
                ones_row_bf = const.tile([1, 128], DBF)
                nc.vector.tensor_copy(ones_row_bf, ones_f[0:1, :])
                eps_c = const.tile([1, 1], F32)
                nc.vector.memset(eps_c, LN_EPS)

                def cload(name, t, shape, dtype=F32):
                    s = const.tile(shape, dtype, name=name)
                    nc.sync.dma_start(out=s, in_=t[tuple(slice(None) for _ in shape)])
                    return s

                cq_sb = const.tile([1, C], DBF, tag="csum_a", name="cqs")
                nc.sync.dma_start(out=cq_sb, in_=cq_t[:, :])
                ckv_sb = const.tile([1, 2 * C], DBF, name="ckvs")
                nc.sync.dma_start(out=ckv_sb[:, 0:C], in_=ck_t[:, :])
                nc.sync.dma_start(out=ckv_sb[:, C:2 * C], in_=cv_t[:, :])
                ck_sb = ckv_sb[:, 0:C]
                cv_pack = ckv_sb[:, C:2 * C]
                bq_sb = cload("bqs", bq_t, [128, 8])
                bk_sb = cload("bks", bk_t, [128, 8])
                bp_sb = cload("bps", bp_t, [128, 8])
                b1_sb = cload("b1s", b1_t, [128, 32])
                b2_sb = cload("b2s", b2_t, [128, 8])
                bvrow_sb = cload("bvrs", bv_t, [1, C], DBF)
                mask_sb = cload("masks", mask_t, [128, 8, 4, QCH], DBF)

                x_own_sb = bigp.tile([128, CH, 512], F32R, tag="xown")
                for ci in range(CH):
                    nc.sync.dma_start(out=x_own_sb[:, ci, :], in_=x_own_t[ci, :, :])

                def ln_group(x_ap, w, out_ap_fn, ps_ln, gi, ma_ap):
                    """Pseudo-LayerNorm of one <=512-token group: writes
                    out = x * rstd (per token) and ma_ap[0, :w] = mu * rstd.
                    The mean subtraction is applied later as a rank-1
                    correction matmul; LN gain/bias are folded into the
                    following weights on the host."""
                    ps_s = ps_ln.tile([1, 512], F32, tag="s", bufs=2, name=f"pss_{it}_{gi}")
                    ps_q = ps_ln.tile([1, 512], F32, tag="q", bufs=2, name=f"psq_{it}_{gi}")
                    for ci in range(CH):
                        nc.tensor.matmul(
                            ps_s[:, :w], ones_col, x_ap[:, ci, :],
                            start=(ci == 0), stop=(ci == CH - 1))
                    for ci in range(CH):
                        sq = tmp.tile([128, 512], F32R, tag="sq", bufs=1,
                                      name=f"sq_{it}_{gi}_{ci}")
                        nc.scalar.activation(sq[:, :w], f32(x_ap[:, ci, :]),
                                             mybir.ActivationFunctionType.Square)
                        nc.tensor.matmul(
                            ps_q[:, :w], ones_col, sq[:, :w],
                            start=(ci == 0), stop=(ci == CH - 1))
                    mu = stat.tile([1, 512], F32, tag="mu", name=f"mu_{it}_{gi}")
                    nc.vector.tensor_scalar_mul(mu[:, :w], ps_s[:, :w], 1.0 / C)
                    ex2 = stat.tile([1, 512], F32, tag="ex2", name=f"ex2_{it}_{gi}")
                    nc.vector.tensor_scalar_mul(ex2[:, :w], ps_q[:, :w], 1.0 / C)
                    var = stat.tile([1, 512], F32, tag="var", name=f"var_{it}_{gi}")
                    nc.scalar.activation(var[:, :w], mu[:, :w],
                                         mybir.ActivationFunctionType.Square)
                    nc.vector.tensor_sub(var[:, :w], ex2[:, :w], var[:, :w])
                    nc.scalar.activation(var[:, :w], var[:, :w],
                                         mybir.ActivationFunctionType.Sqrt,
                                         bias=eps_c[:, :])
                    rstd = stat.tile([1, 512], F32R, tag="ex2", name=f"rstd_{it}_{gi}")
                    nc.vector.reciprocal(rstd[:, :w], var[:, :w])
                    nc.vector.tensor_mul(ma_ap[:, :w], mu[:, :w], f32(rstd[:, :w]))
                    psb_rs = ps_ln.tile([128, 512], F32, tag="brs", bufs=2,
                                        name=f"brs_{it}_{gi}")
                    nc.tensor.matmul(psb_rs[:, :w], ones_row, rstd[:, :w],
                                     start=True, stop=True)
                    for ci in range(CH):
                        nc.vector.tensor_mul(out_ap_fn(ci), f32(x_ap[:, ci, :]),
                                             psb_rs[:, :w])

                h1o = bigp.tile([128, CH, 512], DBF, tag="h1o")
                q_sb = bigp.tile([128, 8, 512], DBF, tag="qsb")
                k_sb = bigp.tile([128, 8, T], DBF, tag="ksb")
                v_sb = bigp.tile([128, 16, H, D + 1], DBF, tag="vsb")
                nc.vector.memset(v_sb[:, :, :, D:D + 1], 1.0)
                if 'kv' in skip:
                    nc.vector.memset(k_sb, 0.01)
                    nc.vector.memset(v_sb[:, :, :, 0:D], 0.01)

                # HBM bounce buffers for the k/v AllGather (bf16).
                # k rows [0,1024): k feature-major [1024 feat, 512 tok]
                # v rows: v token-major [512 tok, 1024 feat], two rows/token.
                # ag_split=False packs both into one buffer (one collective
                # measured cheaper than two back-to-back on this fabric).
                if ag_split:
                    k_bin = dram.tile([1024, 512], DBF)
                    k_bout = dram.tile([4096, 512], DBF)
                    v_bin = dram.tile([1024, 512], DBF)
                    v_bout = dram.tile([4096, 512], DBF)
                    v_rows0 = 0          # v offset within its bounce
                    k_stride, v_stride = 1024, 1024
                else:
                    kv_in = dram.tile([2048, 512], DBF)
                    kv_gather = dram.tile([4 * 2048, 512], DBF)
                    k_bin = kv_in
                    k_bout = kv_gather
                    v_bin = kv_in
                    v_bout = kv_gather
                    v_rows0 = 1024
                    k_stride, v_stride = 2048, 2048

                with (
                    tc.tile_pool(name=f"psln{it}", bufs=1, space="PSUM") as ps_ln,
                    tc.tile_pool(name=f"psacc{it}", bufs=2, space="PSUM") as ps_acc,
                ):
                    # ---- LN1 over own tokens -> h1o ----
                    ma_own = const.tile([1, 512], DBF, name="ma_own")
                    if do('lnown'):
                        ln_group(x_own_sb, 512,
                                 lambda ci: h1o[:, ci, :], ps_ln, "own", ma_own)
                    else:
                        nc.vector.memset(h1o.rearrange('p a b -> p (a b)'), 0.01)
                        nc.vector.memset(ma_own, 0.01)

                    # ---- k for own tokens -> bounce ----
                    k_own = bigp.tile([128, 8, 512], DBF, tag="kown")
                    for mt in ([] if 'kv' in skip else range(8)):
                        wk_sb = wpool.tile([128, CH, 128], DBF, tag="w",
                                           name=f"wk_{it}_{mt}")
                        nc.sync.dma_start(out=wk_sb, in_=wk_t[mt, :, :, :])
                        ps = ps_acc.tile([128, 512], F32, tag="acc",
                                         name=f"psk_{it}_{mt}")
                        for ci in range(CH):
                            nc.tensor.matmul(ps, wk_sb[:, ci, :], h1o[:, ci, :],
                                             start=(ci == 0), stop=False)
                        nc.tensor.matmul(ps, ck_sb[:, 128 * mt:128 * mt + 128],
                                         ma_own, start=False, stop=True)
                        nc.scalar.activation(k_own[:, mt, :], ps,
                                             mybir.ActivationFunctionType.Identity,
                                             bias=bk_sb[:, mt:mt + 1])
                        nc.sync.dma_start(out=k_bin[128 * mt:128 * mt + 128, :],
                                          in_=k_own[:, mt, :])
                    # ---- AllGather k as soon as it is ready (split mode) ----
                    if do('kv') and do('ag') and ag_split:
                        nc.gpsimd.collective_compute(
                            "AllGather",
                            mybir.AluOpType.bypass,
                            replica_groups=[[0, 1, 2, 3], [4, 5, 6, 7]],
                            ins=[k_bin.opt()],
                            outs=[k_bout.opt()],
                        )

                    # ---- v for own tokens (token-major) -> bounce ----
                    wv_sb = bigp.tile([128, CH, C], DBF, tag="wv")
                    if do('kv'):
                        for ci in range(CH):
                            nc.sync.dma_start(out=wv_sb[:, ci, :], in_=wv_t[ci, :, :])
                    v_own = bigp.tile([128, 4, 2, 512], DBF, tag="vown")
                    for tt in ([] if 'kv' in skip else range(4)):
                        for vh in range(2):
                            ps = ps_acc.tile([128, 512], F32, tag="acc",
                                             name=f"psv_{it}_{tt}_{vh}")
                            for ci in range(CH):
                                nc.tensor.matmul(
                                    ps, h1o[:, ci, 128 * tt:128 * tt + 128],
                                    wv_sb[:, ci, 512 * vh:512 * vh + 512],
                                    start=(ci == 0), stop=False)
                            nc.tensor.matmul(
                                ps, ma_own[:, 128 * tt:128 * tt + 128],
                                cv_pack[:, 512 * vh:512 * vh + 512],
                                start=False, stop=False)
                            nc.tensor.matmul(
                                ps, ones_row_bf,
                                bvrow_sb[:, 512 * vh:512 * vh + 512],
                                start=False, stop=True)
                            nc.scalar.activation(
                                v_own[:, tt, vh, :], ps,
                                mybir.ActivationFunctionType.Copy)
                            dst = v_bin[v_rows0 + 256 * tt:v_rows0 + 256 * (tt + 1), :] \
                                .rearrange("(p two) c -> p two c", two=2)[:, vh, :]
                            nc.sync.dma_start(out=dst, in_=v_own[:, tt, vh, :])

                    # ---- AllGather v (split) or fused k+v ----
                    if do('kv') and do('ag'):
                        nc.gpsimd.collective_compute(
                            "AllGather",
                            mybir.AluOpType.bypass,
                            replica_groups=[[0, 1, 2, 3], [4, 5, 6, 7]],
                            ins=[v_bin.opt()],
                            outs=[v_bout.opt()],
                        )

                    # ---- q for own tokens (overlaps the collective) ----
                    for mt in (range(8) if do('q') else []):
                        wq_sb = wpool.tile([128, CH, 128], DBF, tag="w",
                                           name=f"wq_{it}_{mt}")
                        nc.sync.dma_start(out=wq_sb, in_=wq_t[mt, :, :, :])
                        ps = ps_acc.tile([128, 512], F32, tag="acc",
                                         name=f"psq2_{it}_{mt}")
                        for ci in range(CH):
                            nc.tensor.matmul(ps, wq_sb[:, ci, :], h1o[:, ci, :],
                                             start=(ci == 0), stop=False)
                        nc.tensor.matmul(ps, cq_sb[:, 128 * mt:128 * mt + 128],
                                         ma_own, start=False, stop=True)
                        nc.scalar.activation(q_sb[:, mt, :], ps,
                                             mybir.ActivationFunctionType.Identity,
                                             bias=bq_sb[:, mt:mt + 1])
                    if 'q' in skip:
                        nc.vector.memset(q_sb.rearrange('p a b -> p (a b)'), 0.01)

                    # ---- readback of gathered k/v ----
                    if do('kv') and do('rb'):
        # k for the first head-pair, then all v (AV needs it a
                        # few microseconds after the first scores), then the
                        # remaining k head-pairs.
                        def k_rb(mt):
                            for r in range(4):
                                nc.sync.dma_start(
                                    out=k_sb[:, mt, 512 * r:512 * (r + 1)],
                                    in_=k_bout[k_stride * r + 128 * mt:
                                               k_stride * r + 128 * (mt + 1), :])
                        k_rb(0)
                        for r in range(4):
                            for si in range(4):
                                s = 4 * r + si
                                src = v_bout[
                                    v_stride * r + v_rows0 + 256 * si:
                                    v_stride * r + v_rows0 + 256 * (si + 1), :] \
                                    .rearrange("(p two) c -> p (two c)", two=2) \
                                    .rearrange("p (h d) -> p h d", h=H)
                                nc.gpsimd.dma_start(v_sb[:, s, :, 0:D], src)
                        for mt in range(1, 8):
                            k_rb(mt)

                # ---------- attention ----------
                y_sb = bigp.tile([128, 8, 512], F32R, tag="wv")
                with (
                    tc.tile_pool(name=f"psscr{it}", bufs=(2 if G == 2 else 1),
                                 space="PSUM") as ps_scr,
                    tc.tile_pool(name=f"psy{it}", bufs=1, space="PSUM") as ps_y,
                    tc.tile_pool(name=f"psbd{it}", bufs=2, space="PSUM") as ps_bd,
                    tc.tile_pool(name=f"esp{it}", bufs=(6 if G == 2 else 1)) as espool,
                ):
                    att_rts = [] if 'att' in skip else list(range(8))
                    if 'att' in skip:
                        nc.vector.tensor_copy(y_sb.rearrange('p a b -> p (a b)'), ones_f[:, 0:1].broadcast_to((128, CH * 512)))
                    for rt in att_rts:
                        for arm, (nkt, q0) in enumerate([(KT_A, 0), (KT_B, QCH)]):
                            hh = [2 * rt, 2 * rt + 1]
                            psYs = [ps_y.tile([D + 1, QCH], F32, tag=f"y{u}",
                                              name=f"psY_{it}_{rt}_{arm}_{u}")
                                    for u in range(2)]
                            # 2-tile groups, both head-pair members (u) in ONE
                            # shared PSUM tile -> one exp call and at most one
                            # mask call per group (halves ACT/DVE call count).
                            # PSUM tiles rotate (bufs=2) so group g+1's scores
                            # run while group g's exp/AV drain.
                            for g in range(nkt // 2):
                                pall = ps_scr.tile([128, 4, QCH], F32, tag="scr",
                                                   name=f"psS_{it}_{rt}_{arm}_{g}")
                                eall = espool.tile([128, 4, QCH], DBF, tag="es",
                                                   name=f"es_{it}_{rt}_{arm}_{g}")
                                for i in range(2):
                                    kt = 2 * g + i
                                    slot = SLOTA[kt] if arm == 0 else kt
                                    for u in range(2):
                                        po = 64 * u
                                        nc.tensor.matmul(
                                            pall[:, 2 * u + i, :],
                                            k_sb[po:po + 64, rt,
                                                 128 * slot:128 * slot + 128],
                                            q_sb[po:po + 64, rt, q0:q0 + QCH],
                                            start=True, stop=True)
                                nc.scalar.activation(
                                    eall, pall,
                                    mybir.ActivationFunctionType.Exp,
                                    scale=ATT_SCALE)
                                # arm A: mask every group (mask slot g);
                                # arm B: only odd groups (chunk-B tiles) can
                                # be invalid/diagonal (mask slot 4 + g//2)
                                if arm == 0:
                                    nc.vector.tensor_mul(
                                        eall, eall, mask_sb[:, g, :, :])
                                elif g % 2 == 1:
                                    nc.vector.tensor_mul(
                                        eall, eall, mask_sb[:, 4 + g // 2, :, :])
                                # AV for this group, interleaved so PE keeps
                                # busy while ACT runs the next group's exp
                                for u in range(2):
                                    for i in range(2):
                                        kt = 2 * g + i
                                        slot = SLOTA[kt] if arm == 0 else kt
                                        nc.tensor.matmul(
                                            psYs[u], v_sb[:, slot, hh[u], :],
                                            eall[:, 2 * u + i, :],
                                            start=(kt == 0),
                                            stop=(kt == nkt - 1),
                                            skip_group_check=True)
                            for u in range(2):
                                po = 64 * u
                                psY = psYs[u]
                                rd = stat.tile([1, QCH], F32R, tag="mu",
                                               name=f"rd_{it}_{rt}_{arm}_{u}")
                                nc.vector.reciprocal(rd, psY[D:D + 1, :])
                                psD = ps_bd.tile([64, QCH], F32, tag="bd",
                                                 name=f"psD_{it}_{rt}_{arm}_{u}")
                                nc.tensor.matmul(psD, ones_row[:, 0:64], rd,
                                                 start=True, stop=True)
                                rdb = tmp.tile([64, QCH], F32, tag="rdb", bufs=1,
                                               name=f"rdb_{it}_{rt}_{arm}_{u}")
                                nc.scalar.activation(rdb, psD,
                                                     mybir.ActivationFunctionType.Copy)
                                nc.vector.tensor_mul(
                                    y_sb[po:po + 64, rt, q0:q0 + QCH],
                                    psY[0:D, :], rdb)

                # ---------- proj + residual -> x2, LN2, MLP ----------
                x2_sb = bigp.tile([128, CH, 512], F32R, tag="x2")
                h2_sb = bigp.tile([128, CH, 512], F32R, tag="wv")
                hm_sb = bigp.tile([128, 32, 512], DBF, tag="ksb")
                out_sb = bigp.tile([128, CH, 512], F32, tag="vsb")
                with (
                    tc.tile_pool(name=f"psln2{it}", bufs=1, space="PSUM") as ps_ln2,
                    tc.tile_pool(name=f"psacc2{it}", bufs=2, space="PSUM") as ps_acc2,
                ):
                    for mt in (range(8) if do('proj') else []):
                        wp_sb = wpool.tile([128, CH, 128], F32R, tag="w",
                                           name=f"wp_{it}_{mt}")
                        nc.sync.dma_start(out=wp_sb, in_=wp_t[mt, :, :, :])
                        ps = ps_acc2.tile([128, 512], F32, tag="acc",
                                          name=f"psp_{it}_{mt}")
                        for ci in range(CH):
                            nc.tensor.matmul(ps, wp_sb[:, ci, :],
                                             y_sb[:, ci, :],
                                             start=(ci == 0), stop=(ci == CH - 1))
                        nc.vector.scalar_tensor_tensor(
                            out=x2_sb[:, mt, :], in0=ps, scalar=bp_sb[:, mt:mt + 1],
                            in1=f32(x_own_sb[:, mt, :]),
                            op0=mybir.AluOpType.add, op1=mybir.AluOpType.add)
                    if 'proj' in skip:
                        for mt in range(8):
                            nc.vector.tensor_copy(x2_sb[:, mt, :], x_own_sb[:, mt, :])

                    c1_sb = const.tile([1, HID], DBF, tag="csum_a", name="c1s")
                    nc.sync.dma_start(out=c1_sb, in_=c1_t[:, :])
                    ma_x2 = const.tile([1, 512], DBF, name="ma_x2")
                    if do('ln2'):
                        ln_group(x2_sb, 512,
                                 lambda ci: h2_sb[:, ci, :], ps_ln2, "ln2", ma_x2)
                    else:
                        nc.vector.memset(
                            f32(h2_sb.rearrange('p a b -> p (a b)')), 0.01)
                        nc.vector.memset(ma_x2, 0.01)

                    mlp_hts = [] if 'mlp' in skip else list(range(32))
                    if 'mlp' in skip:
                        nc.vector.memset(hm_sb, 0.01)
                    for ht in mlp_hts:
                        w1_sb = wpool.tile([128, CH, 128], F32R, tag="w",
                                           name=f"w1_{it}_{ht}")
                        nc.sync.dma_start(out=w1_sb, in_=w1_t[ht, :, :, :])
                        ps = ps_acc2.tile([128, 512], F32, tag="acc",
                                          name=f"psm1_{it}_{ht}")
                        for ci in range(CH):
                            nc.tensor.matmul(ps, w1_sb[:, ci, :],
                                             h2_sb[:, ci, :],
                                             start=(ci == 0), stop=False)
                        nc.tensor.matmul(ps, c1_sb[:, 128 * ht:128 * ht + 128],
                                         ma_x2, start=False, stop=True)
                        nc.scalar.activation(hm_sb[:, ht, :], ps,
                                             mybir.ActivationFunctionType.Gelu,
                                             bias=b1_sb[:, ht:ht + 1])

                    for mt in range(8):
                        ps = ps_acc2.tile([128, 512], F32, tag="acc",
                                          name=f"psm2_{it}_{mt}")
                        for qq in range(4):
                            w2_sb = wpool.tile([128, 8, 128], DBF, tag="w",
                                               name=f"w2_{it}_{mt}_{qq}")
                            nc.sync.dma_start(out=w2_sb,
                                              in_=w2_t[mt, :, 8 * qq:8 * qq + 8, :])
                            for hc in range(8):
                                g = 8 * qq + hc
                                nc.tensor.matmul(ps, w2_sb[:, hc, :], hm_sb[:, g, :],
                                                 start=(g == 0), stop=(g == 31))
                        nc.vector.scalar_tensor_tensor(
                            out=out_sb[:, mt, :], in0=ps, scalar=b2_sb[:, mt:mt + 1],
                            in1=f32(x2_sb[:, mt, :]),
                            op0=mybir.AluOpType.add, op1=mybir.AluOpType.add)
                    for mt in range(8):
                        nc.sync.dma_start(out=out_t[mt, :, :], in_=out_sb[:, mt, :])

        # Collectives deadlock inside a rolled For_i loop, so repetitions
        # (used only for timing) are unrolled in Python.
        for it in range(reps):
            body(it)

    nc.compile()
    _BUILD_CACHE[key] = nc
    return nc


def _tile_w(w, n_chunks, n_mt):
    """[K, M] -> [n_mt, 128, n_chunks, 128] so each lhsT tile is contiguous."""
    return np.ascontiguousarray(
        w.reshape(n_chunks, 128, n_mt, 128).transpose(2, 1, 0, 3))


def _col8(v):
    """[N*128] -> [128, N] per-partition scalar table."""
    n = v.shape[0] // 128
    return np.ascontiguousarray(v.reshape(n, 128).T)


def slot_pos(s):
    """Start position of gathered k/v slot s (permuted ownership order)."""
    r, rem = divmod(s, 4)
    if rem < 2:
        return 256 * r + 128 * rem            # chunk A of core r
    return 256 * (7 - r) + 128 * (rem - 2)    # chunk B of core r


def make_in_maps(x, w_qkv, b_qkv, w_proj, b_proj, ln1_g, ln1_b, ln2_g, ln2_b,
                 w1, b1, w2, b2):
    f = lambda a: np.asarray(a, np.float32)
    x = f(x)
    w_qkv, b_qkv, w_proj, b_proj = f(w_qkv), f(b_qkv), f(w_proj), f(b_proj)
    w1, b1, w2, b2 = f(w1), f(b1), f(w2), f(b2)
    wq, wk, wv = w_qkv[:, 0:C], w_qkv[:, C:2 * C], w_qkv[:, 2 * C:3 * C]
    # fold LN1 gain into qkv weights and LN1 bias into qkv biases; the
    # per-token mean subtraction becomes a rank-1 correction with the
    # negated column sums (cq/ck/cv).  Same for LN2 into w1/b1.
    wq_e = wq * ln1_g[:, None]
    wk_e = wk * ln1_g[:, None]
    wv_e = wv * ln1_g[:, None]
    bq_e = b_qkv[0:C] + wq.T @ ln1_b
    bk_e = b_qkv[C:2 * C] + wk.T @ ln1_b
    bv_e = b_qkv[2 * C:3 * C] + wv.T @ ln1_b
    w1_e = w1 * ln2_g[:, None]
    b1_e = b1 + w1.T @ ln2_b
    common = {
        "wq": _tile_w(wq_e, CH, 8).astype(BF16),
        "wk": _tile_w(wk_e, CH, 8).astype(BF16),
        "wv": np.ascontiguousarray(wv_e.reshape(CH, 128, C)).astype(BF16),
        "wp": _tile_w(w_proj, CH, 8),
        "w1": _tile_w(w1_e, CH, 32),
        "w2": _tile_w(w2, 32, 8).astype(BF16),
        "bq": _col8(bq_e), "bk": _col8(bk_e),
        "bv": np.ascontiguousarray(bv_e.reshape(1, C)).astype(BF16),
        "bp": _col8(b_proj), "b1": _col8(b1_e), "b2": _col8(b2),
        "cq": (-wq_e.sum(axis=0).reshape(1, C)).astype(BF16),
        "ck": (-wk_e.sum(axis=0).reshape(1, C)).astype(BF16),
        "cv": (-wv_e.sum(axis=0).reshape(1, C)).astype(BF16),
        "c1": (-w1_e.sum(axis=0).reshape(1, HID)).astype(BF16),
    }
    in_maps = []
    kk = np.arange(128)
    qq = np.arange(QCH)
    for c in range(NC):
        seq = c // 4
        j = c % 4
        xf = np.ascontiguousarray(x[seq].T)          # [C, T] feature-major
        qa0, qb0 = QCH * j, QCH * (7 - j)
        x_own = np.concatenate([xf[:, qa0:qa0 + QCH], xf[:, qb0:qb0 + QCH]], axis=1)
        # masks over the gathered (permuted) slot order:
        # rows 0..7: arm A vs the 8 chunk-A slots (SLOTA order)
        # rows 8..15: arm B vs the 8 chunk-B slots (4r+2+ti order)
        masks = np.zeros((16, 128, QCH), np.float32)
        for kt in range(8):
            pos0 = slot_pos(SLOTA[kt])
            masks[kt] = (qa0 + qq[None, :]) >= (pos0 + kk[:, None])
        for mr in range(8):
            r, ti = divmod(mr, 2)
            pos0 = slot_pos(4 * r + 2 + ti)
            masks[8 + mr] = (qb0 + qq[None, :]) >= (pos0 + kk[:, None])
        # expand for the merged-u exp: slot g holds the group's two mask
        # rows duplicated for both head-pair members ((u,i) = 2u+i order).
        mx = np.zeros((8, 4, 128, QCH), np.float32)
        for g in range(4):                       # arm A groups
            mx[g, 0] = mx[g, 2] = masks[2 * g]
            mx[g, 1] = mx[g, 3] = masks[2 * g + 1]
        for gb in range(4):                      # arm B odd groups
            mx[4 + gb, 0] = mx[4 + gb, 2] = masks[8 + 2 * gb]
            mx[4 + gb, 1] = mx[4 + gb, 3] = masks[8 + 2 * gb + 1]
        m = {
            "x_own": np.ascontiguousarray(x_own.reshape(CH, 128, 512)),
            "mask": np.ascontiguousarray(mx.transpose(2, 0, 1, 3)).astype(BF16),
        }
        m.update(common)
        in_maps.append(m)
    return in_maps


def assemble_output(results):
    out = np.zeros((B, T, C), np.float32)
    for c in range(NC):
        seq = c // 4
        j = c % 4
        yf = results[c]["out"].reshape(C, 512)       # feature-major [C, 512]
        qa0, qb0 = QCH * j, QCH * (7 - j)
        out[seq, qa0:qa0 + QCH, :] = yf[:, 0:QCH].T
        out[seq, qb0:qb0 + QCH, :] = yf[:, QCH:2 * QCH].T
    return out


def kernel(**inputs):
    nc = build_kernel(reps=1)
    in_maps = make_in_maps(**inputs)
    res = run_bass_kernel_spmd(nc, in_maps, list(range(NC)))
    return assemble_output(res.results)


# revision 38
# speedup vs baseline: 1.1249x; 1.0573x over previous
"""Trainium2 Bass kernel for a dense transformer block (B=2, T=2048, C=1024, 16 heads).

Strategy (SPMD over 8 cores, one AllGather per group of 4):
  - cores 0-3 handle batch 0, cores 4-7 batch 1 (4 cores per sequence)
  - core with j = core%4 owns two 256-token query chunks: A at 256*j and
    B at 256*(7-j).  Each core computes LN1 + q/k/v ONLY for its own 512
    tokens; k and v (bf16, bias and LN-mean corrections applied) are
    AllGathered across the 4-core sequence group via an HBM bounce
    buffer.  Gathered token order is the ownership permutation
    [c0:A,B | c1:A,B | c2:A,B | c3:A,B]; causality is enforced with
    host-provided multiplicative masks in that permuted order, so the
    instruction stream stays identical on all cores.
  - attention: arm A (low chunk) covers the 8 chunk-A k-slots, arm B
    (high chunk) covers all 16 slots; masks kill invalid/diagonal parts.
  - q is computed while the collective runs; k/v readback is pipelined
    so attention starts as soon as the first head-pair's k has landed.

kernel(**inputs) takes the full unsharded inputs and returns the full
[2, 2048, 1024] output.
"""
import numpy as np
import ml_dtypes

import concourse.bass as bass
import concourse.tile as tile
from concourse import bacc, mybir
from concourse.bass_utils import run_bass_kernel_spmd

BF16 = ml_dtypes.bfloat16
F32 = mybir.dt.float32
F32R = mybir.dt.float32r
DBF = mybir.dt.bfloat16

C = 1024          # embed dim
T = 2048          # seq len
B = 2
H = 16            # heads
D = 64            # head dim
HID = 4096
NC = 8            # cores
CH = C // 128     # 8 channel chunks
QCH = 256         # query chunk width
KT_A = 8          # k-tiles for arm A
KT_B = 16         # k-tiles for arm B
LN_EPS = 1e-5
ATT_SCALE = 1.0 / 8.0   # 1/sqrt(64)

# arm A processes the 8 chunk-A k-slots of the gathered (permuted) kv:
# slot 4*r + ti holds tokens [256*r + 128*ti, +128) of owner core r.
SLOTA = [4 * r + ti for r in range(4) for ti in range(2)]

_BUILD_CACHE = {}


def r32(ap):
    """View an f32 access pattern as float32r for full-rate matmuls."""
    return ap.bitcast(F32R)


def f32(ap):
    """View a float32r access pattern as plain f32 for vector-engine ops."""
    return ap.bitcast(F32)


def build_kernel(reps=1, skip=(), ag_split=True, G=2):
    key = (reps, tuple(skip), ag_split, G)
    if key in _BUILD_CACHE:
        return _BUILD_CACHE[key]
    nc = bacc.Bacc("TRN2", target_bir_lowering=False, debug=False, num_devices=NC)

    # ---- I/O ----
    x_own_t = nc.dram_tensor("x_own", [CH, 128, 512], F32R, kind="ExternalInput")
    wq_t = nc.dram_tensor("wq", [8, 128, CH, 128], DBF, kind="ExternalInput")
    wk_t = nc.dram_tensor("wk", [8, 128, CH, 128], DBF, kind="ExternalInput")
    wv_t = nc.dram_tensor("wv", [CH, 128, C], DBF, kind="ExternalInput")
    wp_t = nc.dram_tensor("wp", [8, 128, CH, 128], F32R, kind="ExternalInput")
    w1_t = nc.dram_tensor("w1", [32, 128, CH, 128], F32R, kind="ExternalInput")
    w2_t = nc.dram_tensor("w2", [8, 128, 32, 128], DBF, kind="ExternalInput")
    bq_t = nc.dram_tensor("bq", [128, 8], F32, kind="ExternalInput")
    bk_t = nc.dram_tensor("bk", [128, 8], F32, kind="ExternalInput")
    bv_t = nc.dram_tensor("bv", [1, C], DBF, kind="ExternalInput")
    bp_t = nc.dram_tensor("bp", [128, 8], F32, kind="ExternalInput")
    b1_t = nc.dram_tensor("b1", [128, 32], F32, kind="ExternalInput")
    b2_t = nc.dram_tensor("b2", [128, 8], F32, kind="ExternalInput")
    cq_t = nc.dram_tensor("cq", [1, C], DBF, kind="ExternalInput")
    ck_t = nc.dram_tensor("ck", [1, C], DBF, kind="ExternalInput")
    cv_t = nc.dram_tensor("cv", [1, C], DBF, kind="ExternalInput")
    c1_t = nc.dram_tensor("c1", [1, HID], DBF, kind="ExternalInput")
    mask_t = nc.dram_tensor("mask", [128, 8, 4, QCH], DBF, kind="ExternalInput")
    out_t = nc.dram_tensor("out", [CH, 128, 512], F32, kind="ExternalOutput")

    with tile.TileContext(nc) as tc, nc.allow_low_precision(
            reason="float32r operands feeding full-rate matmuls"):
        def body(it):
            with (
                tc.tile_pool(name=f"const{it}", bufs=1) as const,
                tc.tile_pool(name=f"bigp{it}", bufs=1) as bigp,
                tc.tile_pool(name=f"wpool{it}", bufs=2) as wpool,
                tc.tile_pool(name=f"stat{it}", bufs=1) as stat,
                tc.tile_pool(name=f"tmp{it}", bufs=2) as tmp,
                tc.tile_pool(name=f"dram{it}", bufs=1, space="DRAM") as dram,
            ):
                def do(name):
                    return name not in skip

                ones_f = const.tile([128, 128], F32)
                nc.vector.memset(ones_f, 1.0)
                ones_col = const.tile([128, 1], F32R)
                nc.vector.tensor_copy(ones_col, ones_f[:, 0:1])
                ones_row = const.tile([1, 128], F32R)
                nc.vector.tensor_copy(ones_row, ones_f[0:1, :])
                ones_row_bf = const.tile([1, 128], DBF)
                nc.vector.tensor_copy(ones_row_bf, ones_f[0:1, :])
                eps_c = const.tile([1, 1], F32)
                nc.vector.memset(eps_c, LN_EPS)

                def cload(name, t, shape, dtype=F32):
                    s = const.tile(shape, dtype, name=name)
                    nc.sync.dma_start(out=s, in_=t[tuple(slice(None) for _ in shape)])
                    return s

                # x_own first in the sync DMA queue: LN1 (the head of the
                # critical path) must not wait behind the 2MB mask load.
                x_own_sb = bigp.tile([128, CH, 512], F32R, tag="xown")
                for ci in range(CH):
                    nc.sync.dma_start(out=x_own_sb[:, ci, :], in_=x_own_t[ci, :, :])

                cq_sb = const.tile([1, C], DBF, tag="csum_a", name="cqs")
                nc.sync.dma_start(out=cq_sb, in_=cq_t[:, :])
                ckv_sb = const.tile([1, 2 * C], DBF, name="ckvs")
                nc.sync.dma_start(out=ckv_sb[:, 0:C], in_=ck_t[:, :])
                nc.sync.dma_start(out=ckv_sb[:, C:2 * C], in_=cv_t[:, :])
                ck_sb = ckv_sb[:, 0:C]
                cv_pack = ckv_sb[:, C:2 * C]
                bq_sb = cload("bqs", bq_t, [128, 8])
                bk_sb = cload("bks", bk_t, [128, 8])
                bp_sb = cload("bps", bp_t, [128, 8])
                b1_sb = cload("b1s", b1_t, [128, 32])
                b2_sb = cload("b2s", b2_t, [128, 8])
                bvrow_sb = cload("bvrs", bv_t, [1, C], DBF)
                mask_sb = cload("masks", mask_t, [128, 8, 4, QCH], DBF)

                def ln_group(x_ap, w, out_ap_fn, ps_ln, gi, ma_ap):
                    """Pseudo-LayerNorm of one <=512-token group: writes
                    out = x * rstd (per token) and ma_ap[0, :w] = mu * rstd.
                    The mean subtraction is applied later as a rank-1
                    correction matmul; LN gain/bias are folded into the
                    following weights on the host."""
                    ps_s = ps_ln.tile([1, 512], F32, tag="s", bufs=2, name=f"pss_{it}_{gi}")
                    ps_q = ps_ln.tile([1, 512], F32, tag="q", bufs=2, name=f"psq_{it}_{gi}")
                    for ci in range(CH):
                        nc.tensor.matmul(
                            ps_s[:, :w], ones_col, x_ap[:, ci, :],
                            start=(ci == 0), stop=(ci == CH - 1))
                    for ci in range(CH):
                        sq = tmp.tile([128, 512], F32R, tag="sq", bufs=1,
                                      name=f"sq_{it}_{gi}_{ci}")
                        nc.scalar.activation(sq[:, :w], f32(x_ap[:, ci, :]),
                                             mybir.ActivationFunctionType.Square)
                        nc.tensor.matmul(
                            ps_q[:, :w], ones_col, sq[:, :w],
                            start=(ci == 0), stop=(ci == CH - 1))
                    mu = stat.tile([1, 512], F32, tag="mu", name=f"mu_{it}_{gi}")
                    nc.vector.tensor_scalar_mul(mu[:, :w], ps_s[:, :w], 1.0 / C)
                    ex2 = stat.tile([1, 512], F32, tag="ex2", name=f"ex2_{it}_{gi}")
                    nc.vector.tensor_scalar_mul(ex2[:, :w], ps_q[:, :w], 1.0 / C)
                    var = stat.tile([1, 512], F32, tag="var", name=f"var_{it}_{gi}")
                    nc.scalar.activation(var[:, :w], mu[:, :w],
                                         mybir.ActivationFunctionType.Square)
                    nc.vector.tensor_sub(var[:, :w], ex2[:, :w], var[:, :w])
                    nc.scalar.activation(var[:, :w], var[:, :w],
                                         mybir.ActivationFunctionType.Sqrt,
                                         bias=eps_c[:, :])
                    rstd = stat.tile([1, 512], F32R, tag="ex2", name=f"rstd_{it}_{gi}")
                    nc.vector.reciprocal(rstd[:, :w], var[:, :w])
                    nc.vector.tensor_mul(ma_ap[:, :w], mu[:, :w], f32(rstd[:, :w]))
                    psb_rs = ps_ln.tile([128, 512], F32, tag="brs", bufs=2,
                                        name=f"brs_{it}_{gi}")
                    nc.tensor.matmul(psb_rs[:, :w], ones_row, rstd[:, :w],
                                     start=True, stop=True)
                    for ci in range(CH):
                        nc.vector.tensor_mul(out_ap_fn(ci), f32(x_ap[:, ci, :]),
                                             psb_rs[:, :w])

                h1o = bigp.tile([128, CH, 512], DBF, tag="h1o")
                q_sb = bigp.tile([128, 8, 512], DBF, tag="qsb")
                k_sb = bigp.tile([128, 8, T], DBF, tag="ksb")
                v_sb = bigp.tile([128, 16, H, D + 1], DBF, tag="vsb")
                nc.vector.memset(v_sb[:, :, :, D:D + 1], 1.0)
                if 'kv' in skip:
                    nc.vector.memset(k_sb, 0.01)
                    nc.vector.memset(v_sb[:, :, :, 0:D], 0.01)

                # HBM bounce buffers for the k/v AllGather (bf16).
                # k rows [0,1024): k feature-major [1024 feat, 512 tok]
                # v rows: v token-major [512 tok, 1024 feat], two rows/token.
                # ag_split=False packs both into one buffer (one collective
                # measured cheaper than two back-to-back on this fabric).
                if ag_split:
                    k_bin = dram.tile([1024, 512], DBF)
                    k_bout = dram.tile([4096, 512], DBF)
                    v_bin = dram.tile([1024, 512], DBF)
                    v_bout = dram.tile([4096, 512], DBF)
                    v_rows0 = 0          # v offset within its bounce
                    k_stride, v_stride = 1024, 1024
                else:
                    kv_in = dram.tile([2048, 512], DBF)
                    kv_gather = dram.tile([4 * 2048, 512], DBF)
                    k_bin = kv_in
                    k_bout = kv_gather
                    v_bin = kv_in
                    v_bout = kv_gather
                    v_rows0 = 1024
                    k_stride, v_stride = 2048, 2048

                with (
                    tc.tile_pool(name=f"psln{it}", bufs=1, space="PSUM") as ps_ln,
                    tc.tile_pool(name=f"psacc{it}", bufs=2, space="PSUM") as ps_acc,
                ):
                    # ---- LN1 over own tokens -> h1o ----
                    ma_own = const.tile([1, 512], DBF, name="ma_own")
                    if do('lnown'):
                        ln_group(x_own_sb, 512,
                                 lambda ci: h1o[:, ci, :], ps_ln, "own", ma_own)
                    else:
                        nc.vector.memset(h1o.rearrange('p a b -> p (a b)'), 0.01)
                        nc.vector.memset(ma_own, 0.01)

                    # ---- k for own tokens -> bounce ----
                    k_own = bigp.tile([128, 8, 512], DBF, tag="kown")
                    for mt in ([] if 'kv' in skip else range(8)):
                        wk_sb = wpool.tile([128, CH, 128], DBF, tag="w",
                                           name=f"wk_{it}_{mt}")
                        nc.sync.dma_start(out=wk_sb, in_=wk_t[mt, :, :, :])
                        ps = ps_acc.tile([128, 512], F32, tag="acc",
                                         name=f"psk_{it}_{mt}")
                        for ci in range(CH):
                            nc.tensor.matmul(ps, wk_sb[:, ci, :], h1o[:, ci, :],
                                             start=(ci == 0), stop=False)
                        nc.tensor.matmul(ps, ck_sb[:, 128 * mt:128 * mt + 128],
                                         ma_own, start=False, stop=True)
                        nc.scalar.activation(k_own[:, mt, :], ps,
                                             mybir.ActivationFunctionType.Identity,
                                             bias=bk_sb[:, mt:mt + 1])
                        nc.sync.dma_start(out=k_bin[128 * mt:128 * mt + 128, :],
                                          in_=k_own[:, mt, :])
                    # ---- AllGather k as soon as it is ready (split mode) ----
                    if do('kv') and do('ag') and ag_split:
                        nc.gpsimd.collective_compute(
                            "AllGather",
                            mybir.AluOpType.bypass,
                            replica_groups=[[0, 1, 2, 3], [4, 5, 6, 7]],
                            ins=[k_bin.opt()],
                            outs=[k_bout.opt()],
                        )

                    # ---- v for own tokens (token-major) -> bounce ----
                    wv_sb = bigp.tile([128, CH, C], DBF, tag="wv")
                    if do('kv'):
                        for ci in range(CH):
                            nc.sync.dma_start(out=wv_sb[:, ci, :], in_=wv_t[ci, :, :])
                    v_own = bigp.tile([128, 4, 2, 512], DBF, tag="vown")
                    for tt in ([] if 'kv' in skip else range(4)):
                        for vh in range(2):
                            ps = ps_acc.tile([128, 512], F32, tag="acc",
                                             name=f"psv_{it}_{tt}_{vh}")
                            for ci in range(CH):
                                nc.tensor.matmul(
                                    ps, h1o[:, ci, 128 * tt:128 * tt + 128],
                                    wv_sb[:, ci, 512 * vh:512 * vh + 512],
                                    start=(ci == 0), stop=False)
                            nc.tensor.matmul(
                                ps, ma_own[:, 128 * tt:128 * tt + 128],
                                cv_pack[:, 512 * vh:512 * vh + 512],
                                start=False, stop=False)
                            nc.tensor.matmul(
                                ps, ones_row_bf,
                                bvrow_sb[:, 512 * vh:512 * vh + 512],
                                start=False, stop=True)
                            nc.scalar.activation(
                                v_own[:, tt, vh, :], ps,
                                mybir.ActivationFunctionType.Copy)
                            dst = v_bin[v_rows0 + 256 * tt:v_rows0 + 256 * (tt + 1), :] \
                                .rearrange("(p two) c -> p two c", two=2)[:, vh, :]
                            nc.sync.dma_start(out=dst, in_=v_own[:, tt, vh, :])

                    # ---- AllGather v (split) or fused k+v ----
                    if do('kv') and do('ag'):
                        nc.gpsimd.collective_compute(
                            "AllGather",
                            mybir.AluOpType.bypass,
                            replica_groups=[[0, 1, 2, 3], [4, 5, 6, 7]],
                            ins=[v_bin.opt()],
                            outs=[v_bout.opt()],
                        )

                    # ---- q for own tokens (overlaps the collective) ----
                    for mt in (range(8) if do('q') else []):
                        wq_sb = wpool.tile([128, CH, 128], DBF, tag="w",
                                           name=f"wq_{it}_{mt}")
                        nc.sync.dma_start(out=wq_sb, in_=wq_t[mt, :, :, :])
                        ps = ps_acc.tile([128, 512], F32, tag="acc",
                                         name=f"psq2_{it}_{mt}")
                        for ci in range(CH):
                            nc.tensor.matmul(ps, wq_sb[:, ci, :], h1o[:, ci, :],
                                             start=(ci == 0), stop=False)
                        nc.tensor.matmul(ps, cq_sb[:, 128 * mt:128 * mt + 128],
                                         ma_own, start=False, stop=True)
                        nc.scalar.activation(q_sb[:, mt, :], ps,
                                             mybir.ActivationFunctionType.Identity,
                                             bias=bq_sb[:, mt:mt + 1])
                    if 'q' in skip:
                        nc.vector.memset(q_sb.rearrange('p a b -> p (a b)'), 0.01)

                    # ---- readback of gathered k/v ----
                    if do('kv') and do('rb'):
        # k for the first head-pair, then all v (AV needs it a
                        # few microseconds after the first scores), then the
                        # remaining k head-pairs.
                        def k_rb(mt):
                            # one 3D-AP DMA per head-pair: all 4 source
                            # blocks (1KB lines) in a single transfer
                            src = k_bout.rearrange("(r q) c -> q r c", r=4)[
                                128 * mt:128 * (mt + 1), :, :]
                            nc.sync.dma_start(
                                out=k_sb[:, mt, :].rearrange(
                                    "p (r c) -> p r c", r=4),
                                in_=src)
                        k_rb(0)
                        for r in range(4):
                            for si in range(4):
                                s = 4 * r + si
                                src = v_bout[
                                    v_stride * r + v_rows0 + 256 * si:
                                    v_stride * r + v_rows0 + 256 * (si + 1), :] \
                                    .rearrange("(p two) c -> p (two c)", two=2) \
                                    .rearrange("p (h d) -> p h d", h=H)
                                nc.gpsimd.dma_start(v_sb[:, s, :, 0:D], src)
                        for mt in range(1, 8):
                            k_rb(mt)

                # ---------- attention ----------
                y_sb = bigp.tile([128, 8, 512], F32R, tag="wv")
                with (
                    tc.tile_pool(name=f"psscr{it}", bufs=(2 if G == 2 else 1),
                                 space="PSUM") as ps_scr,
                    tc.tile_pool(name=f"psy{it}", bufs=1, space="PSUM") as ps_y,
                    tc.tile_pool(name=f"psbd{it}", bufs=2, space="PSUM") as ps_bd,
                    tc.tile_pool(name=f"esp{it}", bufs=(6 if G == 2 else 1)) as espool,
                ):
                    att_rts = [] if 'att' in skip else list(range(8))
                    if 'att' in skip:
                        nc.vector.tensor_copy(y_sb.rearrange('p a b -> p (a b)'), ones_f[:, 0:1].broadcast_to((128, CH * 512)))
                    for rt in att_rts:
                        for arm, (nkt, q0) in enumerate([(KT_A, 0), (KT_B, QCH)]):
                            hh = [2 * rt, 2 * rt + 1]
                            # separate PSUM banks per head-pair member: PSUM
                            # zeroing on start=True is bank-granular, so two
                            # interleaved accumulation groups must not share
                            # a bank
                            psYs = [ps_y.tile([D + 1, QCH], F32, tag=f"y{u}",
                                              name=f"psY_{it}_{rt}_{arm}_{u}")
                                    for u in range(2)]
                            # 2-tile groups, both head-pair members (u) in ONE
                            # shared PSUM tile -> one exp call and at most one
                            # mask call per group (halves ACT/DVE call count).
                            # PSUM tiles rotate (bufs=2) so group g+1's scores
                            # run while group g's exp/AV drain.
                            for g in range(nkt // 2):
                                pall = ps_scr.tile([128, 4, QCH], F32, tag="scr",
                                                   name=f"psS_{it}_{rt}_{arm}_{g}")
                                eall = espool.tile([128, 4, QCH], DBF, tag="es",
                                                   name=f"es_{it}_{rt}_{arm}_{g}")
                                for i in range(2):
                                    kt = 2 * g + i
                                    slot = SLOTA[kt] if arm == 0 else kt
                                    for u in range(2):
                                        po = 64 * u
                                        nc.tensor.matmul(
                                            pall[:, 2 * u + i, :],
                                            k_sb[po:po + 64, rt,
                                                 128 * slot:128 * slot + 128],
                                            q_sb[po:po + 64, rt, q0:q0 + QCH],
                                            start=True, stop=True)
                                nc.scalar.activation(
                                    eall, pall,
                                    mybir.ActivationFunctionType.Exp,
                                    scale=ATT_SCALE)
                                # arm A: mask every group (mask slot g);
                                # arm B: only odd groups (chunk-B tiles) can
                                # be invalid/diagonal (mask slot 4 + g//2)
                                if arm == 0:
                                    nc.vector.tensor_mul(
                                        eall, eall, mask_sb[:, g, :, :])
                                elif g % 2 == 1:
                                    nc.vector.tensor_mul(
                                        eall, eall, mask_sb[:, 4 + g // 2, :, :])
                                # AV for this group, interleaved so PE keeps
                                # busy while ACT runs the next group's exp
                                for u in range(2):
                                    for i in range(2):
                                        kt = 2 * g + i
                                        slot = SLOTA[kt] if arm == 0 else kt
                                        nc.tensor.matmul(
                                            psYs[u], v_sb[:, slot, hh[u], :],
                                            eall[:, 2 * u + i, :],
                                            start=(kt == 0),
                                            stop=(kt == nkt - 1),
                                            skip_group_check=True)
                            for u in range(2):
                                po = 64 * u
                                psY = psYs[u]
                                rd = stat.tile([1, QCH], F32R, tag="mu",
                                               name=f"rd_{it}_{rt}_{arm}_{u}")
                                nc.vector.reciprocal(rd, psY[D:D + 1, :])
                                psD = ps_bd.tile([64, QCH], F32, tag="bd",
                                                 name=f"psD_{it}_{rt}_{arm}_{u}")
                                nc.tensor.matmul(psD, ones_row[:, 0:64], rd,
                                                 start=True, stop=True)
                                rdb = tmp.tile([64, QCH], F32, tag="rdb", bufs=1,
                                               name=f"rdb_{it}_{rt}_{arm}_{u}")
                                nc.scalar.activation(rdb, psD,
                                                     mybir.ActivationFunctionType.Copy)
                                nc.vector.tensor_mul(
                                    y_sb[po:po + 64, rt, q0:q0 + QCH],
                                    psY[0:D, :], rdb)

                # ---------- proj + residual -> x2, LN2, MLP ----------
                x2_sb = bigp.tile([128, CH, 512], F32R, tag="x2")
                h2_sb = bigp.tile([128, CH, 512], F32R, tag="wv")
                hm_sb = bigp.tile([128, 32, 512], DBF, tag="ksb")
                out_sb = bigp.tile([128, CH, 512], F32, tag="vsb")
                with (
                    tc.tile_pool(name=f"psln2{it}", bufs=1, space="PSUM") as ps_ln2,
                    tc.tile_pool(name=f"psacc2{it}", bufs=2, space="PSUM") as ps_acc2,
                ):
                    for mt in (range(8) if do('proj') else []):
                        wp_sb = wpool.tile([128, CH, 128], F32R, tag="w",
                                           name=f"wp_{it}_{mt}")
                        nc.sync.dma_start(out=wp_sb, in_=wp_t[mt, :, :, :])
                        ps = ps_acc2.tile([128, 512], F32, tag="acc",
                                          name=f"psp_{it}_{mt}")
                        for ci in range(CH):
                            nc.tensor.matmul(ps, wp_sb[:, ci, :],
                                             y_sb[:, ci, :],
                                             start=(ci == 0), stop=(ci == CH - 1))
                        nc.vector.scalar_tensor_tensor(
                            out=x2_sb[:, mt, :], in0=ps, scalar=bp_sb[:, mt:mt + 1],
                            in1=f32(x_own_sb[:, mt, :]),
                            op0=mybir.AluOpType.add, op1=mybir.AluOpType.add)
                    if 'proj' in skip:
                        for mt in range(8):
                            nc.vector.tensor_copy(x2_sb[:, mt, :], x_own_sb[:, mt, :])

                    c1_sb = const.tile([1, HID], DBF, tag="csum_a", name="c1s")
                    nc.sync.dma_start(out=c1_sb, in_=c1_t[:, :])
                    ma_x2 = const.tile([1, 512], DBF, name="ma_x2")
                    if do('ln2'):
                        ln_group(x2_sb, 512,
                                 lambda ci: h2_sb[:, ci, :], ps_ln2, "ln2", ma_x2)
                    else:
                        nc.vector.memset(
                            f32(h2_sb.rearrange('p a b -> p (a b)')), 0.01)
                        nc.vector.memset(ma_x2, 0.01)

                    mlp_hts = [] if 'mlp' in skip else list(range(32))
                    if 'mlp' in skip:
                        nc.vector.memset(hm_sb, 0.01)
                    for ht in mlp_hts:
                        w1_sb = wpool.tile([128, CH, 128], F32R, tag="w",
                                           name=f"w1_{it}_{ht}")
                        nc.sync.dma_start(out=w1_sb, in_=w1_t[ht, :, :, :])
                        ps = ps_acc2.tile([128, 512], F32, tag="acc",
                                          name=f"psm1_{it}_{ht}")
                        for ci in range(CH):
                            nc.tensor.matmul(ps, w1_sb[:, ci, :],
                                             h2_sb[:, ci, :],
                                             start=(ci == 0), stop=False)
                        nc.tensor.matmul(ps, c1_sb[:, 128 * ht:128 * ht + 128],
                                         ma_x2, start=False, stop=True)
                        nc.scalar.activation(hm_sb[:, ht, :], ps,
                                             mybir.ActivationFunctionType.Gelu,
                                             bias=b1_sb[:, ht:ht + 1])

                    for mt in range(8):
                        ps = ps_acc2.tile([128, 512], F32, tag="acc",
                                          name=f"psm2_{it}_{mt}")
                        for qq in range(4):
                            w2_sb = wpool.tile([128, 8, 128], DBF, tag="w",
                                               name=f"w2_{it}_{mt}_{qq}")
                            nc.sync.dma_start(out=w2_sb,
                                              in_=w2_t[mt, :, 8 * qq:8 * qq + 8, :])
                            for hc in range(8):
                                g = 8 * qq + hc
                                nc.tensor.matmul(ps, w2_sb[:, hc, :], hm_sb[:, g, :],
                                                 start=(g == 0), stop=(g == 31))
                        nc.vector.scalar_tensor_tensor(
                            out=out_sb[:, mt, :], in0=ps, scalar=b2_sb[:, mt:mt + 1],
                            in1=f32(x2_sb[:, mt, :]),
                            op0=mybir.AluOpType.add, op1=mybir.AluOpType.add)
                    for mt in range(8):
                        nc.sync.dma_start(out=out_t[mt, :, :], in_=out_sb[:, mt, :])

        # Collectives deadlock inside a rolled For_i loop, so repetitions
        # (used only for timing) are unrolled in Python.
        for it in range(reps):
            body(it)

    nc.compile()
    _BUILD_CACHE[key] = nc
    return nc


def _tile_w(w, n_chunks, n_mt):
    """[K, M] -> [n_mt, 128, n_chunks, 128] so each lhsT tile is contiguous."""
    return np.ascontiguousarray(
        w.reshape(n_chunks, 128, n_mt, 128).transpose(2, 1, 0, 3))


def _col8(v):
    """[N*128] -> [128, N] per-partition scalar table."""
    n = v.shape[0] // 128
    return np.ascontiguousarray(v.reshape(n, 128).T)


def slot_pos(s):
    """Start position of gathered k/v slot s (permuted ownership order)."""
    r, rem = divmod(s, 4)
    if rem < 2:
        return 256 * r + 128 * rem            # chunk A of core r
    return 256 * (7 - r) + 128 * (rem - 2)    # chunk B of core r


def make_in_maps(x, w_qkv, b_qkv, w_proj, b_proj, ln1_g, ln1_b, ln2_g, ln2_b,
                 w1, b1, w2, b2):
    f = lambda a: np.asarray(a, np.float32)
    x = f(x)
    w_qkv, b_qkv, w_proj, b_proj = f(w_qkv), f(b_qkv), f(w_proj), f(b_proj)
    w1, b1, w2, b2 = f(w1), f(b1), f(w2), f(b2)
    wq, wk, wv = w_qkv[:, 0:C], w_qkv[:, C:2 * C], w_qkv[:, 2 * C:3 * C]
    # fold LN1 gain into qkv weights and LN1 bias into qkv biases; the
    # per-token mean subtraction becomes a rank-1 correction with the
    # negated column sums (cq/ck/cv).  Same for LN2 into w1/b1.
    wq_e = wq * ln1_g[:, None]
    wk_e = wk * ln1_g[:, None]
    wv_e = wv * ln1_g[:, None]
    bq_e = b_qkv[0:C] + wq.T @ ln1_b
    bk_e = b_qkv[C:2 * C] + wk.T @ ln1_b
    bv_e = b_qkv[2 * C:3 * C] + wv.T @ ln1_b
    w1_e = w1 * ln2_g[:, None]
    b1_e = b1 + w1.T @ ln2_b
    common = {
        "wq": _tile_w(wq_e, CH, 8).astype(BF16),
        "wk": _tile_w(wk_e, CH, 8).astype(BF16),
        "wv": np.ascontiguousarray(wv_e.reshape(CH, 128, C)).astype(BF16),
        "wp": _tile_w(w_proj, CH, 8),
        "w1": _tile_w(w1_e, CH, 32),
        "w2": _tile_w(w2, 32, 8).astype(BF16),
        "bq": _col8(bq_e), "bk": _col8(bk_e),
        "bv": np.ascontiguousarray(bv_e.reshape(1, C)).astype(BF16),
        "bp": _col8(b_proj), "b1": _col8(b1_e), "b2": _col8(b2),
        "cq": (-wq_e.sum(axis=0).reshape(1, C)).astype(BF16),
        "ck": (-wk_e.sum(axis=0).reshape(1, C)).astype(BF16),
        "cv": (-wv_e.sum(axis=0).reshape(1, C)).astype(BF16),
        "c1": (-w1_e.sum(axis=0).reshape(1, HID)).astype(BF16),
    }
    in_maps = []
    kk = np.arange(128)
    qq = np.arange(QCH)
    for c in range(NC):
        seq = c // 4
        j = c % 4
        xf = np.ascontiguousarray(x[seq].T)          # [C, T] feature-major
        qa0, qb0 = QCH * j, QCH * (7 - j)
        x_own = np.concatenate([xf[:, qa0:qa0 + QCH], xf[:, qb0:qb0 + QCH]], axis=1)
        # masks over the gathered (permuted) slot order:
        # rows 0..7: arm A vs the 8 chunk-A slots (SLOTA order)
        # rows 8..15: arm B vs the 8 chunk-B slots (4r+2+ti order)
        masks = np.zeros((16, 128, QCH), np.float32)
        for kt in range(8):
            pos0 = slot_pos(SLOTA[kt])
            masks[kt] = (qa0 + qq[None, :]) >= (pos0 + kk[:, None])
        for mr in range(8):
            r, ti = divmod(mr, 2)
            pos0 = slot_pos(4 * r + 2 + ti)
            masks[8 + mr] = (qb0 + qq[None, :]) >= (pos0 + kk[:, None])
        # expand for the merged-u exp: slot g holds the group's two mask
        # rows duplicated for both head-pair members ((u,i) = 2u+i order).
        mx = np.zeros((8, 4, 128, QCH), np.float32)
        for g in range(4):                       # arm A groups
            mx[g, 0] = mx[g, 2] = masks[2 * g]
            mx[g, 1] = mx[g, 3] = masks[2 * g + 1]
        for gb in range(4):                      # arm B odd groups
            mx[4 + gb, 0] = mx[4 + gb, 2] = masks[8 + 2 * gb]
            mx[4 + gb, 1] = mx[4 + gb, 3] = masks[8 + 2 * gb + 1]
        m = {
            "x_own": np.ascontiguousarray(x_own.reshape(CH, 128, 512)),
            "mask": np.ascontiguousarray(mx.transpose(2, 0, 1, 3)).astype(BF16),
        }
        m.update(common)
        in_maps.append(m)
    return in_maps


def assemble_output(results):
    out = np.zeros((B, T, C), np.float32)
    for c in range(NC):
        seq = c // 4
        j = c % 4
        yf = results[c]["out"].reshape(C, 512)       # feature-major [C, 512]
        qa0, qb0 = QCH * j, QCH * (7 - j)
        out[seq, qa0:qa0 + QCH, :] = yf[:, 0:QCH].T
        out[seq, qb0:qb0 + QCH, :] = yf[:, QCH:2 * QCH].T
    return out


def kernel(**inputs):
    nc = build_kernel(reps=1)
    in_maps = make_in_maps(**inputs)
    res = run_bass_kernel_spmd(nc, in_maps, list(range(NC)))
    return assemble_output(res.results)
